# revision 9
# baseline (speedup 1.0000x reference)
"""Multi-head attention (B=2, S=2048, E=1024, H=16, causal, relative bias)
as a Bass/Tile kernel on 8 Trainium2 NeuronCores.

Sharding: core c handles batch b = c//4 and head group hg = c%4 (4 heads).
All device-side math runs in "transposed" space (k on partitions) so no
on-chip transposes are needed:
  - Q^T, K^T [d, s] come straight out of the projections (out = W^T.T @ x^T)
  - scoresT[k, q] = K^T.T-slice @ Q^T-slice  (contraction over d=64)
  - rel-bias + causal mask are one precomputed "stair" matrix per head,
    added into the scores PSUM via an identity matmul on the TensorEngine
  - softmax denominator comes free from a ones-column appended to V in the
    attn@V matmul (PSUM row 64 accumulates sum_k exp)
  - 1/denom via exp(-log(d)) on the scalar engine (vector.reciprocal is slow)
  - attn written to HBM transposed (attn_t[h, k, q]); host transposes back
Matmuls run in float32r (full PE rate, ~1.5e-4 rel err). The upper-triangle
(fully masked) blocks are never computed or written; output buffers are
zero-initialized by the runtime so they read back as exact 0.0, matching
softmax(-inf) = 0 in the reference.
"""

import numpy as np

import concourse.bass as bass  # noqa: F401  (bass types used via tile/bacc)
import concourse.tile as tile
from concourse import bacc, mybir
from concourse.bass_utils import run_bass_kernel_spmd

B, S, E, H = 2, 2048, 1024, 16
D = E // H          # 64
NH = 4              # heads per core
HS = NH * D         # 256 = per-core head-slice width
MAX_REL = 512
N_CORES = 8

SB = 512            # q/s block
KB = 128            # k block
NSB = S // SB       # 4
NKB = S // KB       # 16
NEB = E // 128      # 8 e-blocks
STAIR_U = 1536      # stair free size
MASK_VAL = -1e9

F32 = mybir.dt.float32
F32R = mybir.dt.float32r

_NC_CACHE = {}


def _build_nc():
    """Build + compile the SPMD kernel (same NEFF for all 8 cores)."""
    nc = bacc.Bacc("TRN2", target_bir_lowering=False, debug=False,
                   num_devices=N_CORES)

    # ---- I/O ----
    xq = nc.dram_tensor("xq_t", [E, S], F32R, kind="ExternalInput")
    xk = nc.dram_tensor("xk_t", [E, S], F32R, kind="ExternalInput")
    xv = nc.dram_tensor("xv_t", [E, S], F32R, kind="ExternalInput")
    wq = nc.dram_tensor("wq_t", [E, HS], F32R, kind="ExternalInput")
    wk = nc.dram_tensor("wk_t", [E, HS], F32R, kind="ExternalInput")
    wv = nc.dram_tensor("wv_t", [E, HS], F32R, kind="ExternalInput")
    wo = nc.dram_tensor("wo_t", [HS, E], F32R, kind="ExternalInput")
    bq2 = nc.dram_tensor("bq2", [128, 2], F32, kind="ExternalInput")
    bk2 = nc.dram_tensor("bk2", [128, 2], F32, kind="ExternalInput")
    bv1 = nc.dram_tensor("bv1", [1, HS], F32R, kind="ExternalInput")
    stair_d = nc.dram_tensor("stair", [NH, 128, STAIR_U], F32R,
                             kind="ExternalInput")
    satb_d = nc.dram_tensor("satb", [128, NH], F32, kind="ExternalInput")
    ident_d = nc.dram_tensor("ident", [128, 128], F32R, kind="ExternalInput")
    ones_d = nc.dram_tensor("onesb", [128, 128], F32R, kind="ExternalInput")
    vone_d = nc.dram_tensor("vone", [128, NH], F32R, kind="ExternalInput")

    attn_t = nc.dram_tensor("attn_t", [NH, S, S], F32R, kind="ExternalOutput")
    out_t = nc.dram_tensor("out_t", [E, S], F32, kind="ExternalOutput")

    with tile.TileContext(nc) as tc:
        with (
            tc.tile_pool(name="const", bufs=1) as constp,
            tc.tile_pool(name="wop", bufs=1) as wop,
            tc.tile_pool(name="persist", bufs=1) as persist,
        ):
            # ---- constants ----
            ident = constp.tile([128, 128], F32R, tag="ident")
            nc.sync.dma_start(ident[:], ident_d[:, :])
            onesb = constp.tile([128, 128], F32R, tag="onesb")
            nc.sync.dma_start(onesb[:], ones_d[:, :])
            bq_sb = constp.tile([128, 2], F32, tag="bq")
            nc.sync.dma_start(bq_sb[:], bq2[:, :])
            bk_sb = constp.tile([128, 2], F32, tag="bk")
            nc.sync.dma_start(bk_sb[:], bk2[:, :])
            bv_sb = constp.tile([1, HS], F32R, tag="bv")
            nc.sync.dma_start(bv_sb[:], bv1[:, :])
            satb = constp.tile([128, NH], F32, tag="satb")
            nc.sync.dma_start(satb[:], satb_d[:, :])

            wo_sb = [wop.tile([D, E], F32R, tag=f"wo{h}", name=f"wo_sb{h}")
                     for h in range(NH)]
            for h in range(NH):
                nc.sync.dma_start(wo_sb[h][:], wo[D * h:D * (h + 1), :])

            # ---- persistent activations ----
            q_t = [persist.tile([128, S], F32R, tag=f"q{t}", name=f"q_t{t}")
                   for t in range(2)]
            k_t = [persist.tile([128, S], F32R, tag=f"k{t}", name=f"k_t{t}")
                   for t in range(2)]
            # v_aug[j]: [128, NH*65]; per head h: cols h*65..h*65+64 = V, +1s
            v_aug = [persist.tile([128, NH * 65], F32R, tag=f"v{j}", name=f"v_aug{j}")
                     for j in range(NKB)]
            o_t = [persist.tile([D, S], F32R, tag=f"o{h}", name=f"o_t{h}")
                   for h in range(NH)]

            for j in range(NKB):
                nc.sync.dma_start(
                    v_aug[j].rearrange("p (h c) -> p h c", c=65)[:, :, 64:65],
                    vone_d.ap().rearrange("p (h c) -> p h c", c=1))

            # ---- Phase A: projections ----
            with (
                tc.tile_pool(name="wqkv", bufs=1) as wqkv,
                tc.tile_pool(name="xstage", bufs=2) as xstage,
                tc.tile_pool(name="pps", bufs=2, space="PSUM") as pps,
            ):
                wq_sb = wqkv.tile([128, NEB, HS], F32R, tag="wq")
                nc.sync.dma_start(wq_sb[:],
                                  wq.ap().rearrange("(eb p) o -> p eb o", p=128))
                wk_sb = wqkv.tile([128, NEB, HS], F32R, tag="wk")
                nc.sync.dma_start(wk_sb[:],
                                  wk.ap().rearrange("(eb p) o -> p eb o", p=128))
                wv_sb = wqkv.tile([128, NEB, HS], F32R, tag="wv")
                nc.sync.dma_start(wv_sb[:],
                                  wv.ap().rearrange("(eb p) o -> p eb o", p=128))
                for sb in range(NSB):
                    s0 = sb * SB
                    xq_st = xstage.tile([128, NEB, SB], F32R, tag="xst",
                                        name="xq_st")
                    nc.sync.dma_start(
                        xq_st[:],
                        xq.ap().rearrange("(eb p) s -> p eb s", p=128)[:, :, s0:s0 + SB])
                    xk_st = xstage.tile([128, NEB, SB], F32R, tag="xst",
                                        name="xk_st")
                    nc.sync.dma_start(
                        xk_st[:],
                        xk.ap().rearrange("(eb p) s -> p eb s", p=128)[:, :, s0:s0 + SB])
                    xv_st = xstage.tile([128, NEB, SB], F32R, tag="xst",
                                        name="xv_st")
                    nc.sync.dma_start(
                        xv_st[:],
                        xv.ap().rearrange("(eb p) s -> p eb s", p=128)[:, :, s0:s0 + SB])

                    # Q^T and K^T: out[o-tile 128, s 512]
                    for t in range(2):
                        ps = pps.tile([128, SB], F32, tag="proj")
                        for eb in range(NEB):
                            nc.tensor.matmul(
                                ps[:], wq_sb[:, eb, 128 * t:128 * (t + 1)],
                                xq_st[:, eb, :],
                                start=(eb == 0), stop=(eb == NEB - 1))
                        nc.scalar.activation(
                            q_t[t][:, s0:s0 + SB], ps[:],
                            mybir.ActivationFunctionType.Identity,
                            bias=bq_sb[:, t:t + 1], scale=1.0)
                        ps = pps.tile([128, SB], F32, tag="proj")
                        for eb in range(NEB):
                            nc.tensor.matmul(
                                ps[:], wk_sb[:, eb, 128 * t:128 * (t + 1)],
                                xk_st[:, eb, :],
                                start=(eb == 0), stop=(eb == NEB - 1))
                        nc.scalar.activation(
                            k_t[t][:, s0:s0 + SB], ps[:],
                            mybir.ActivationFunctionType.Identity,
                            bias=bk_sb[:, t:t + 1], scale=1.0)

                    # V natural: out[s-subtile 128, d 256] + ones-row bias
                    for jj in range(SB // 128):
                        j = sb * (SB // 128) + jj
                        ps = pps.tile([128, HS], F32, tag="vproj")
                        for eb in range(NEB):
                            nc.tensor.matmul(
                                ps[:], xv_st[:, eb, 128 * jj:128 * (jj + 1)],
                                wv_sb[:, eb, :],
                                start=(eb == 0), stop=False)
                        nc.tensor.matmul(ps[:], onesb[0:1, :],
                                         bv_sb[0:1, :],
                                         start=False, stop=True,
                                         skip_group_check=True)
                        nc.scalar.copy(
                            v_aug[j].rearrange("p (h c) -> p h c", c=65)[:, :, 0:64],
                            ps.rearrange("p (h c) -> p h c", c=64))

            # ---- Phase B: attention ----
            with (
                tc.tile_pool(name="bsb", bufs=1) as bsb,
                tc.tile_pool(name="expp", bufs=1) as expp,
                tc.tile_pool(name="small", bufs=2) as small,
                tc.tile_pool(name="sps", bufs=4, space="PSUM") as sps,
                tc.tile_pool(name="oaps", bufs=2, space="PSUM") as oaps,
                tc.tile_pool(name="reps", bufs=2, space="PSUM") as reps,
            ):
                stair = bsb.tile([128, NH, STAIR_U], F32R, tag="stair")
                nc.sync.dma_start(stair[:],
                                  stair_d.ap().rearrange("h p u -> p h u"))
                for h in range(NH):
                    t, ph = h // 2, (h % 2) * 64
                    for qb in range(NSB):
                        q0 = qb * SB
                        nkb = 4 * qb + 4
                        es = expp.tile([128, NKB, SB], F32R, tag="es")
                        oa = oaps.tile([65, SB], F32, tag="oa")
                        for kb in range(nkb):
                            k0 = kb * KB
                            delta = q0 - k0
                            sp = sps.tile([128, SB], F32, tag="sc")
                            saturated = delta > MAX_REL
                            nc.tensor.matmul(
                                sp[:],
                                k_t[t][ph:ph + 64, k0:k0 + KB],
                                q_t[t][ph:ph + 64, q0:q0 + SB],
                                start=True, stop=saturated)
                            if not saturated:
                                u0 = delta + MAX_REL
                                nc.tensor.matmul(
                                    sp[:], ident[:],
                                    stair[:, h, u0:u0 + SB],
                                    start=False, stop=True,
                                    skip_group_check=True)
                                bias = 0.0
                            else:
                                bias = satb[:, h:h + 1]
                            nc.scalar.activation(
                                es[:, kb, :], sp[:],
                                mybir.ActivationFunctionType.Exp,
                                bias=bias, scale=1.0)
                            nc.tensor.matmul(
                                oa[:],
                                v_aug[kb].rearrange(
                                    "p (hh c) -> p hh c", c=65)[:, h, :],
                                es[:, kb, :],
                                start=(kb == 0), stop=(kb == nkb - 1),
                                skip_group_check=True)

                        # reciprocal of denominator: exp(-log(denom))
                        rc = small.tile([128, SB], F32, tag="rc")
                        nc.scalar.activation(rc[64:65, :], oa[64:65, :],
                                             mybir.ActivationFunctionType.Ln,
                                             bias=0.0, scale=1.0)
                        rc2 = small.tile([128, SB], F32R, tag="rc2")
                        nc.scalar.activation(rc2[64:65, :], rc[64:65, :],
                                             mybir.ActivationFunctionType.Exp,
                                             bias=0.0, scale=-1.0)
                        rep = reps.tile([128, SB], F32, tag="rep")
                        nc.tensor.matmul(rep[:], onesb[64:65, :],
                                         rc2[64:65, :], start=True, stop=True)
                        rep_sb = small.tile([128, SB], F32, tag="repsb")
                        nc.scalar.copy(rep_sb[:], rep[:])

                        # normalize out^T for this head/q-block
                        nc.vector.tensor_mul(
                            o_t[h][:, q0:q0 + SB], oa[0:64, :], rep_sb[0:64, :])
                        # normalize attn blocks in place, then one DMA out
                        for kb in range(nkb):
                            nc.vector.tensor_mul(
                                es[:, kb, :], es[:, kb, :], rep_sb[:])
                        nc.sync.dma_start(
                            attn_t.ap()[h].rearrange(
                                "(kb p) q -> p kb q", p=128)[:, 0:nkb, q0:q0 + SB],
                            es[:, 0:nkb, :])

            # ---- Phase C: output projection (partial, transposed) ----
            with (
                tc.tile_pool(name="ostage", bufs=2) as ostage,
                tc.tile_pool(name="cps", bufs=2, space="PSUM") as cps,
            ):
                for m in range(E // 128):
                    o0 = m * 128
                    ost = ostage.tile([128, S], F32, tag="ost")
                    for sb in range(NSB):
                        s0 = sb * SB
                        ps = cps.tile([128, SB], F32, tag="ops")
                        for h in range(NH):
                            nc.tensor.matmul(
                                ps[:], wo_sb[h][:, o0:o0 + 128],
                                o_t[h][:, s0:s0 + SB],
                                start=(h == 0), stop=(h == NH - 1))
                        nc.scalar.copy(ost[:, s0:s0 + SB], ps[:])
                    nc.sync.dma_start(out_t[o0:o0 + 128, :], ost[:])

    nc.compile()
    return nc


def _prep_core_inputs(inputs, c):
    """Host-side sharding for core c (batch c//4, head group c%4)."""
    b = c // 4
    hg = c % 4
    sl = slice(HS * hg, HS * (hg + 1))

    Wq, Wk, Wv, Wo = inputs["Wq"], inputs["Wk"], inputs["Wv"], inputs["Wo"]
    scale = np.float32(D ** -0.5)

    xq_t = np.ascontiguousarray(inputs["query"][b].T)
    xk_t = np.ascontiguousarray(inputs["key"][b].T)
    xv_t = np.ascontiguousarray(inputs["value"][b].T)

    wq_t = np.ascontiguousarray(Wq[sl, :].T * scale)    # scale folded into Q
    wk_t = np.ascontiguousarray(Wk[sl, :].T)
    wv_t = np.ascontiguousarray(Wv[sl, :].T)
    wo_t = np.ascontiguousarray(Wo[:, sl].T)

    bq_c = (inputs["bq"][sl] * scale).reshape(2, 128).T
    bk_c = inputs["bk"][sl].reshape(2, 128).T
    bv_c = inputs["bv"][sl].reshape(1, HS)

    table = inputs["rel_bias_table"]                     # [1025, 16]
    heads = np.arange(NH * hg, NH * (hg + 1))
    # stair[h, i, u] = (u-i < 512) ? MASK : table[clip(u-i,0,1024), head]
    u_i = np.arange(STAIR_U)[None, :] - np.arange(128)[:, None]   # [128,1536]
    idx = np.clip(u_i, 0, 2 * MAX_REL)
    stair = np.where(u_i[None] < MAX_REL, np.float32(MASK_VAL),
                     table[idx, :][:, :, heads].transpose(2, 0, 1))
    stair = np.ascontiguousarray(stair.astype(np.float32))
    satb = np.broadcast_to(table[2 * MAX_REL, heads][None, :],
                           (128, NH)).copy()

    return {
        "xq_t": np.ascontiguousarray(xq_t, np.float32),
        "xk_t": np.ascontiguousarray(xk_t, np.float32),
        "xv_t": np.ascontiguousarray(xv_t, np.float32),
        "wq_t": wq_t.astype(np.float32), "wk_t": wk_t.astype(np.float32),
        "wv_t": wv_t.astype(np.float32), "wo_t": wo_t.astype(np.float32),
        "bq2": np.ascontiguousarray(bq_c, np.float32),
        "bk2": np.ascontiguousarray(bk_c, np.float32),
        "bv1": np.ascontiguousarray(bv_c, np.float32),
        "stair": stair,
        "satb": np.ascontiguousarray(satb, np.float32),
        "ident": np.eye(128, dtype=np.float32),
        "onesb": np.ones((128, 128), np.float32),
        "vone": np.ones((128, NH), np.float32),
    }


def kernel(**inputs):
    key = "nc"
    if key not in _NC_CACHE:
        _NC_CACHE[key] = _build_nc()
    nc = _NC_CACHE[key]

    in_maps = [_prep_core_inputs(inputs, c) for c in range(N_CORES)]
    res = run_bass_kernel_spmd(nc, in_maps, core_ids=list(range(N_CORES)))

    # ---- host-side unshard ----
    out = np.empty((B, S, E), np.float32)
    attn = np.zeros((B, H, S, S), np.float32)
    bo = np.asarray(inputs["bo"], np.float32)
    for b in range(B):
        acc = res.results[4 * b]["out_t"].copy()
        for hg in range(1, 4):
            acc += res.results[4 * b + hg]["out_t"]
        out[b] = acc.T
        out[b] += bo[None, :]
    for c in range(N_CORES):
        b, hg = c // 4, c % 4
        at = res.results[c]["attn_t"]                    # [4, S, S] (k, q)
        for h in range(NH):
            attn[b, NH * hg + h] = at[h].T
    return out, attn
